# revision 1
# baseline (speedup 1.0000x reference)
"""Bilinear sampler (spatial transformer) TRN2 Bass kernel.

Contract: kernel(inputs=[128, 196614] fp32) -> [128, 256, 256, 3] fp32.
Shards batch over 8 NeuronCores (16 images each). Per image on-device:
  - compute affine grid X = t00*j + t01*i + cx, Y likewise (ACT/DVE)
  - floors, bilinear weights with out-of-bounds masking (DVE)
  - build a row-pair interleaved copy of the image in DRAM scratch
    (site l = y*256+x holds rows y and y+1 of column x: 6 floats), so one
    contiguous 12-float fetch at offset 6*l yields the whole 2x2x3 patch
  - per pixel-column instruction: [P,1] indirect DMA gather (128 patches)
  - weighted blend of the 4 corners (DVE), DMA out
"""
import os
import sys

sys.path.insert(0, "/opt/trn_rl_repo")

import numpy as np

import concourse.bacc as bacc
import concourse.bass as bass
import concourse.mybir as mybir
import concourse.tile as tile
from concourse.bass_utils import run_bass_kernel_spmd

P = 128
H = W = 256
C = 3
IMG_ELS = H * W * C            # 196608
ROW_ELS = W * C                # 768
PW = (H * W) // P              # 512 pixels per partition per image
N_CORES = 8
IMGS = 16                      # images per core

F32 = mybir.dt.float32
I32 = mybir.dt.int32
ALU = mybir.AluOpType

_cached = {}


def _build(n_imgs):
    nc = bacc.Bacc("TRN2", target_bir_lowering=False, debug=False,
                   enable_asserts=False, num_devices=1, num_swdge_queues=2)
    inp = nc.dram_tensor("inp", [n_imgs, 6 + IMG_ELS], F32, kind="ExternalInput")
    xg_d = nc.dram_tensor("xg", [P, PW], F32, kind="ExternalInput")
    yg_d = nc.dram_tensor("yg", [P, PW], F32, kind="ExternalInput")
    cst_d = nc.dram_tensor("cst", [2, 4], F32, kind="ExternalInput")
    out_d = nc.dram_tensor("out", [n_imgs, H * W * C], F32, kind="ExternalOutput")
    idups = [nc.dram_tensor(f"idup{b}", [H * W, 6], F32) for b in range(n_imgs)]
    scr = nc.dram_tensor("scr", [n_imgs, 8], F32)

    with tile.TileContext(nc) as tc:
        with (
            tc.tile_pool(name="const", bufs=1) as cpool,
            tc.tile_pool(name="work", bufs=1) as wp,
            tc.tile_pool(name="gath", bufs=2) as gpool,
            tc.tile_pool(name="offp", bufs=2) as opool,
        ):
            xg = cpool.tile([P, PW], F32)
            nc.sync.dma_start(xg[:], xg_d[:, :])
            yg = cpool.tile([P, PW], F32)
            nc.sync.dma_start(yg[:], yg_d[:, :])
            cst = cpool.tile([2, 4], F32)
            nc.sync.dma_start(cst[:], cst_d[:, :])

            for b in range(n_imgs):
                # ---- affine params: [2,3] theta rows; cx/cy = 127.5*(t2+1-t0-t1)
                th = wp.tile([2, 3], F32)
                nc.sync.dma_start(th[:], bass.AP(inp, b * (6 + IMG_ELS), [[3, 2], [1, 3]]))
                m = wp.tile([2, 3], F32)
                nc.vector.tensor_tensor(out=m[:], in0=th[:], in1=cst[:, 0:3], op=ALU.mult)
                s = wp.tile([2, 1], F32)
                nc.vector.tensor_reduce(out=s[:], in_=m[:], axis=mybir.AxisListType.X, op=ALU.add)
                pr = wp.tile([2, 4], F32)
                nc.vector.tensor_copy(out=pr[:, 0:3], in_=th[:])
                nc.vector.tensor_scalar(out=pr[:, 3:4], in0=s[:], scalar1=127.5,
                                        scalar2=None, op0=ALU.add)
                nc.sync.dma_start(bass.AP(scr, b * 8, [[4, 2], [1, 4]]), pr[:])
                thb = wp.tile([P, 8], F32)
                nc.sync.dma_start(thb[:], bass.AP(scr, b * 8, [[0, P], [1, 8]]))
                # thb cols: 0=t00 1=t01 2=t02(unused) 3=cx 4=t10 5=t11 6=t12 7=cy

                # ---- build row-pair interleaved image copy in DRAM
                it = wp.tile([P, 1536], F32)
                nc.sync.dma_start(it[:], bass.AP(inp, b * (6 + IMG_ELS) + 6,
                                                 [[1536, P], [1, 1536]]))
                hal = wp.tile([P, ROW_ELS], F32)
                nc.sync.dma_start(hal[0:127, :],
                                  bass.AP(inp, b * (6 + IMG_ELS) + 6 + 1536,
                                          [[1536, 127], [1, ROW_ELS]]))
                nc.sync.dma_start(hal[127:128, :],
                                  bass.AP(inp, b * (6 + IMG_ELS) + 6 + IMG_ELS - ROW_ELS,
                                          [[ROW_ELS, 1], [1, ROW_ELS]]))
                d2 = wp.tile([P, PW, 6], F32)
                it3 = it[:].rearrange("p (w c) -> p w c", c=3)
                nc.vector.tensor_copy(out=d2[:, :, 0:3], in_=it3)
                nc.vector.tensor_copy(out=d2[:, 0:256, 3:6],
                                      in_=it[:, ROW_ELS:1536].rearrange("p (w c) -> p w c", c=3))
                nc.vector.tensor_copy(out=d2[:, 256:512, 3:6],
                                      in_=hal[:].rearrange("p (w c) -> p w c", c=3))
                nc.sync.dma_start(idups[b][:, :], d2[:])

                # ---- grid coords
                X = wp.tile([P, PW], F32)
                nc.vector.tensor_scalar(out=X[:], in0=xg[:], scalar1=thb[:, 0:1],
                                        scalar2=None, op0=ALU.mult)
                X2 = wp.tile([P, PW], F32)
                nc.vector.scalar_tensor_tensor(out=X2[:], in0=yg[:], scalar=thb[:, 1:2],
                                               in1=X[:], op0=ALU.mult, op1=ALU.add)
                nc.vector.tensor_scalar(out=X[:], in0=X2[:], scalar1=thb[:, 3:4],
                                        scalar2=None, op0=ALU.add)
                Y = wp.tile([P, PW], F32)
                nc.vector.tensor_scalar(out=Y[:], in0=xg[:], scalar1=thb[:, 4:5],
                                        scalar2=None, op0=ALU.mult)
                Y2 = wp.tile([P, PW], F32)
                nc.vector.scalar_tensor_tensor(out=Y2[:], in0=yg[:], scalar=thb[:, 5:6],
                                               in1=Y[:], op0=ALU.mult, op1=ALU.add)
                nc.vector.tensor_scalar(out=Y[:], in0=Y2[:], scalar1=thb[:, 7:8],
                                        scalar2=None, op0=ALU.add)

                # ---- floor via int truncation + correction
                def floor_of(src, nm):
                    ti = wp.tile([P, PW], I32, tag=f"fl_i{nm}")
                    nc.vector.tensor_copy(out=ti[:], in_=src[:])
                    tf = wp.tile([P, PW], F32, tag=f"fl_f{nm}")
                    nc.vector.tensor_copy(out=tf[:], in_=ti[:])
                    gt = wp.tile([P, PW], F32, tag=f"fl_g{nm}")
                    nc.vector.tensor_tensor(out=gt[:], in0=tf[:], in1=src[:], op=ALU.is_gt)
                    fl = wp.tile([P, PW], F32, tag=f"fl_o{nm}")
                    nc.vector.tensor_tensor(out=fl[:], in0=tf[:], in1=gt[:], op=ALU.subtract)
                    return fl

                xf = floor_of(X, "x")
                yf = floor_of(Y, "y")

                # ---- weights with OOB masks
                fx = wp.tile([P, PW], F32)
                nc.vector.tensor_tensor(out=fx[:], in0=X[:], in1=xf[:], op=ALU.subtract)
                fy = wp.tile([P, PW], F32)
                nc.vector.tensor_tensor(out=fy[:], in0=Y[:], in1=yf[:], op=ALU.subtract)
                al = wp.tile([P, PW], F32)
                nc.vector.tensor_scalar(out=al[:], in0=fx[:], scalar1=-1.0, scalar2=1.0,
                                        op0=ALU.mult, op1=ALU.add)
                ga = wp.tile([P, PW], F32)
                nc.vector.tensor_scalar(out=ga[:], in0=fy[:], scalar1=-1.0, scalar2=1.0,
                                        op0=ALU.mult, op1=ALU.add)
                mgx = wp.tile([P, PW], F32)
                nc.vector.tensor_scalar(out=mgx[:], in0=xf[:], scalar1=0.0, scalar2=None,
                                        op0=ALU.is_ge)
                mx = wp.tile([P, PW], F32)
                nc.vector.scalar_tensor_tensor(out=mx[:], in0=xf[:], scalar=254.0,
                                               in1=mgx[:], op0=ALU.is_le, op1=ALU.mult)
                mgy = wp.tile([P, PW], F32)
                nc.vector.tensor_scalar(out=mgy[:], in0=yf[:], scalar1=0.0, scalar2=None,
                                        op0=ALU.is_ge)
                my = wp.tile([P, PW], F32)
                nc.vector.scalar_tensor_tensor(out=my[:], in0=yf[:], scalar=254.0,
                                               in1=mgy[:], op0=ALU.is_le, op1=ALU.mult)
                A = wp.tile([P, PW], F32)
                nc.vector.tensor_tensor(out=A[:], in0=al[:], in1=mx[:], op=ALU.mult)
                Bw = wp.tile([P, PW], F32)
                nc.vector.tensor_tensor(out=Bw[:], in0=fx[:], in1=mx[:], op=ALU.mult)
                Cw = wp.tile([P, PW], F32)
                nc.vector.tensor_tensor(out=Cw[:], in0=ga[:], in1=my[:], op=ALU.mult)
                Dw = wp.tile([P, PW], F32)
                nc.vector.tensor_tensor(out=Dw[:], in0=fy[:], in1=my[:], op=ALU.mult)
                w00 = wp.tile([P, PW], F32)
                nc.vector.tensor_tensor(out=w00[:], in0=Cw[:], in1=A[:], op=ALU.mult)
                w10 = wp.tile([P, PW], F32)
                nc.vector.tensor_tensor(out=w10[:], in0=Dw[:], in1=A[:], op=ALU.mult)
                w01 = wp.tile([P, PW], F32)
                nc.vector.tensor_tensor(out=w01[:], in0=Cw[:], in1=Bw[:], op=ALU.mult)
                w11 = wp.tile([P, PW], F32)
                nc.vector.tensor_tensor(out=w11[:], in0=Dw[:], in1=Bw[:], op=ALU.mult)

                # ---- gather offsets: site = clamp(yf,0,254)*256 + clamp(xf,0,254)
                xc = wp.tile([P, PW], F32)
                nc.vector.tensor_scalar(out=xc[:], in0=xf[:], scalar1=0.0, scalar2=254.0,
                                        op0=ALU.max, op1=ALU.min)
                yc = wp.tile([P, PW], F32)
                nc.vector.tensor_scalar(out=yc[:], in0=yf[:], scalar1=0.0, scalar2=254.0,
                                        op0=ALU.max, op1=ALU.min)
                lf = wp.tile([P, PW], F32)
                nc.vector.scalar_tensor_tensor(out=lf[:], in0=yc[:], scalar=256.0,
                                               in1=xc[:], op0=ALU.mult, op1=ALU.add)
                off = opool.tile([P, PW], I32)
                nc.vector.tensor_copy(out=off[:], in_=lf[:])

                # ---- per-column [P,1] patch gathers, alternating SWDGE queues
                g = gpool.tile([P, PW, 12], F32)
                for w in range(PW):
                    inst = nc.gpsimd.indirect_dma_start(
                        out=g[:, w, :], out_offset=None,
                        in_=idups[b][:, :],
                        in_offset=bass.IndirectOffsetOnAxis(ap=off[:, w:w + 1], axis=0))
                    if w % 2:
                        inst.ins.queue = "qPoolDynamic1"

                # ---- blend: slices (r,s): 0:3=(0,0) 3:6=(1,0) 6:9=(0,1) 9:12=(1,1)
                def bc3(t):
                    return bass.AP(t.tensor, t.offset, list(t.ap) + [[0, 3]])

                t0 = wp.tile([P, PW, 3], F32)
                nc.vector.tensor_tensor(out=t0[:], in0=g[:, :, 0:3], in1=bc3(w00[:]), op=ALU.mult)
                t1 = wp.tile([P, PW, 3], F32)
                nc.vector.tensor_tensor(out=t1[:], in0=g[:, :, 3:6], in1=bc3(w10[:]), op=ALU.mult)
                t2 = wp.tile([P, PW, 3], F32)
                nc.vector.tensor_tensor(out=t2[:], in0=g[:, :, 6:9], in1=bc3(w01[:]), op=ALU.mult)
                t3 = wp.tile([P, PW, 3], F32)
                nc.vector.tensor_tensor(out=t3[:], in0=g[:, :, 9:12], in1=bc3(w11[:]), op=ALU.mult)
                nc.vector.tensor_tensor(out=t0[:], in0=t0[:], in1=t1[:], op=ALU.add)
                nc.vector.tensor_tensor(out=t2[:], in0=t2[:], in1=t3[:], op=ALU.add)
                ob = wp.tile([P, PW, 3], F32)
                nc.vector.tensor_tensor(out=ob[:], in0=t0[:], in1=t2[:], op=ALU.add)
                nc.sync.dma_start(bass.AP(out_d, b * IMG_ELS, [[PW * 3, P], [1, PW * 3]]),
                                  ob[:])
    nc.compile()
    return nc


def _consts():
    # pixel (p, w): global l = p*PW + w ; j = l % 256 ; i = l // 256
    l = np.arange(P * PW).reshape(P, PW)
    xg = (l % 256).astype(np.float32)
    yg = (l // 256).astype(np.float32)
    cst = np.tile(np.array([-127.5, -127.5, 127.5, 0.0], np.float32), (2, 1))
    return xg, yg, cst


IMGS_PER_LAUNCH = 16


def kernel(inputs: np.ndarray) -> np.ndarray:
    inputs = np.ascontiguousarray(inputs, dtype=np.float32)
    assert inputs.shape == (128, 6 + IMG_ELS)
    npl = IMGS_PER_LAUNCH
    if npl not in _cached:
        _cached[npl] = _build(npl)
    nc = _cached[npl]
    xg, yg, cst = _consts()
    trace = bool(os.environ.get("BILIN_TRACE"))
    if trace:
        try:  # NTFF trace hook is missing from this image's antenv; install shim
            import antenv.axon_hooks  # noqa: F401
        except ImportError:
            try:
                import types
                from trn_agent_boot.trn_boot import _ntff_profile_via_ctypes
                hook = _ntff_profile_via_ctypes("/opt/axon/libaxon_pjrt.so")
                mod = types.ModuleType("antenv.axon_hooks")
                mod.get_axon_ntff_profile_hook = lambda: hook
                sys.modules["antenv.axon_hooks"] = mod
            except Exception:
                trace = False
    out = np.empty((128, H, W, C), np.float32)
    total_ns = 0
    n_launches = IMGS // npl
    for k in range(n_launches):
        in_maps = []
        for c in range(N_CORES):
            lo = c * IMGS + k * npl
            in_maps.append(dict(inp=np.ascontiguousarray(inputs[lo:lo + npl]),
                                xg=xg, yg=yg, cst=cst))
        res = run_bass_kernel_spmd(nc, in_maps, core_ids=list(range(N_CORES)),
                                   trace=trace and k == 0)
        if trace and k == 0 and res.exec_time_ns is not None:
            total_ns = res.exec_time_ns * n_launches
        for c in range(N_CORES):
            lo = c * IMGS + k * npl
            out[lo:lo + npl] = res.results[c]["out"].reshape(npl, H, W, C)
    if trace:
        print(f"HW exec time: {total_ns} ns")
    return out



# revision 13
# speedup vs baseline: 1.4208x; 1.4208x over previous
"""Bilinear sampler (spatial transformer) TRN2 Bass kernel.

Contract: kernel(inputs=[128, 196614] fp32) -> [128, 256, 256, 3] fp32.
Shards batch over 8 NeuronCores (16 images each).

Mode 'dg' (default): per image, build a 256B-aligned site table in DRAM
(site = y0*64 + x0//4 holds rows y0,y0+1 x cols 4m..4m+4, 30 used floats
padded to 64), then fetch one 256B block per output pixel with
InstDMAGatherAnt (vectorized int16 indices, ~0.34ns/desc gen vs ~1.3us
per 128-desc INDIRECT1D instruction).  The variable within-block
position (s = x0 mod 4) is resolved by a one-hot mask e1[s] contracted
against the 4 candidate patch windows during the blend.

Mode 'i1': original per-column [P,1] indirect-DMA gather baseline.
"""
import os
import sys

sys.path.insert(0, "/opt/trn_rl_repo")

import numpy as np

import concourse.bacc as bacc
import concourse.bass as bass
import concourse.mybir as mybir
import concourse.tile as tile
from concourse.bass_utils import run_bass_kernel_spmd

P = 128
H = W = 256
C = 3
IMG_ELS = H * W * C            # 196608
ROW_ELS = W * C                # 768
PW = (H * W) // P              # 512 pixels per partition per image
N_CORES = 8
IMGS = 16                      # images per core

F32 = mybir.dt.float32
I32 = mybir.dt.int32
I16 = mybir.dt.int16
ALU = mybir.AluOpType
AX = mybir.AxisListType

GMODE = os.environ.get("BILIN_GMODE", "dg")
NOGATHER = bool(os.environ.get("BILIN_NOGATHER"))
NOSHUF = bool(os.environ.get("BILIN_NOSHUF"))
NOEXT = bool(os.environ.get("BILIN_NOEXT"))
Q0 = bool(os.environ.get("BILIN_Q0"))
NQ = int(os.environ.get("BILIN_NQ", "2"))
NCHUNK = int(os.environ.get("BILIN_NCHUNK", "16"))  # gather chunks/image;
# 4096 idxs -> 2x257 ring descriptors, fits the 1024-desc SWDGE carveout
CPX = PW // NCHUNK             # 64 w-columns per chunk
NSITE = 16384                  # sites per image: 256 rowpairs x 64 col-blocks
SITE_ELS = 64                  # 256B per site

_cached = {}


def _ap(t, extra_off, dims):
    """Strided view of a tile AP: keep partition dim, custom free dims."""
    return bass.AP(t.tensor, t.offset + extra_off, [list(t.ap[0])] + dims)


def _build_dg(n_imgs):
    nc = bacc.Bacc("TRN2", target_bir_lowering=False, debug=False,
                   enable_asserts=False, num_devices=1, num_swdge_queues=NQ)
    inp = nc.dram_tensor("inp", [n_imgs, 6 + IMG_ELS], F32, kind="ExternalInput")
    xg_d = nc.dram_tensor("xg", [P, PW], F32, kind="ExternalInput")
    yg_d = nc.dram_tensor("yg", [P, PW], F32, kind="ExternalInput")
    cst_d = nc.dram_tensor("cst", [2, 4], F32, kind="ExternalInput")
    i4_d = nc.dram_tensor("i4", [P, 4], F32, kind="ExternalInput")
    out_d = nc.dram_tensor("out", [n_imgs, IMG_ELS], F32, kind="ExternalOutput")
    idups = [nc.dram_tensor(f"idup{b}", [NSITE, SITE_ELS], F32)
             for b in range(n_imgs)]
    didx = nc.dram_tensor("didx", [n_imgs, P * PW], I16)
    scr = nc.dram_tensor("scr", [n_imgs, 8], F32)

    with tile.TileContext(nc) as tc:
        with (
            tc.tile_pool(name="const", bufs=1) as cpool,
            tc.tile_pool(name="img", bufs=2) as ipool,
            tc.tile_pool(name="d2p", bufs=1) as d2pool,
            tc.tile_pool(name="work", bufs=1) as wp,
            tc.tile_pool(name="gath", bufs=2) as gpool,
            tc.tile_pool(name="tmpp", bufs=1) as tpool,
            tc.tile_pool(name="idxp", bufs=2) as xpool,
            tc.tile_pool(name="ext", bufs=2) as epool,
            tc.tile_pool(name="outp", bufs=2) as opool,
        ):
            xg = cpool.tile([P, PW], F32)
            nc.sync.dma_start(xg[:], xg_d[:, :])
            yg = cpool.tile([P, PW], F32)
            nc.sync.dma_start(yg[:], yg_d[:, :])
            cst = cpool.tile([2, 4], F32)
            nc.sync.dma_start(cst[:], cst_d[:, :])
            i4 = cpool.tile([P, 4], F32)
            nc.sync.dma_start(i4[:], i4_d[:, :])

            for b in range(n_imgs):
                # ---- affine params: thb cols 0=t00 1=t01 3=cx 4=t10 5=t11 7=cy
                th = wp.tile([2, 3], F32)
                nc.sync.dma_start(th[:], bass.AP(inp, b * (6 + IMG_ELS), [[3, 2], [1, 3]]))
                m = wp.tile([2, 3], F32)
                nc.vector.tensor_tensor(out=m[:], in0=th[:], in1=cst[:, 0:3], op=ALU.mult)
                s = wp.tile([2, 1], F32)
                nc.vector.tensor_reduce(out=s[:], in_=m[:], axis=AX.X, op=ALU.add)
                pr = wp.tile([2, 4], F32)
                nc.vector.tensor_copy(out=pr[:, 0:3], in_=th[:])
                nc.vector.tensor_scalar(out=pr[:, 3:4], in0=s[:], scalar1=127.5,
                                        scalar2=None, op0=ALU.add)
                nc.sync.dma_start(bass.AP(scr, b * 8, [[4, 2], [1, 4]]), pr[:])
                thb = wp.tile([P, 8], F32)
                nc.sync.dma_start(thb[:], bass.AP(scr, b * 8, [[0, P], [1, 8]]))

                # ---- image loads: it rows 2p,2p+1; hal row 2p+2 (padded)
                it = ipool.tile([P, 1544], F32)
                hal = ipool.tile([P, 776], F32)
                if b < 2:
                    nc.vector.memset(it[:], 0.0)
                    nc.vector.memset(hal[:], 0.0)
                nc.sync.dma_start(it[:, 0:1536],
                                  bass.AP(inp, b * (6 + IMG_ELS) + 6,
                                          [[1536, P], [1, 1536]]))
                nc.sync.dma_start(hal[0:127, 0:ROW_ELS],
                                  bass.AP(inp, b * (6 + IMG_ELS) + 6 + 1536,
                                          [[1536, 127], [1, ROW_ELS]]))
                nc.sync.dma_start(hal[127:128, 0:ROW_ELS],
                                  bass.AP(inp, b * (6 + IMG_ELS) + 6 + IMG_ELS - ROW_ELS,
                                          [[ROW_ELS, 1], [1, ROW_ELS]]))

                # ---- d2: [2 rp, 64 m, 64] per partition; site (2p+rp)*64+m
                # holds offset 6c+3sr+ch = row (2p+rp+sr), col (4m+c), c<5
                d2 = d2pool.tile([P, 2 * 64 * SITE_ELS], F32)
                if b < 2:
                    nc.vector.memset(d2[:], 0.0)
                d2a = d2[:]
                ita = it[:]
                hala = hal[:]
                for rp in range(2):
                    for sr in range(2):
                        row = rp + sr           # 0,1,2 relative to 2p
                        if row < 2:
                            src = _ap(ita, row * ROW_ELS, [[12, 64], [3, 5], [1, 3]])
                        else:
                            src = _ap(hala, 0, [[12, 64], [3, 5], [1, 3]])
                        dst = _ap(d2a, rp * 64 * SITE_ELS + 3 * sr,
                                  [[SITE_ELS, 64], [6, 5], [1, 3]])
                        nc.vector.tensor_copy(out=dst, in_=src)
                nc.sync.dma_start(idups[b][:, :], d2[:])

                # ---- grid coords (pixel units)
                X = wp.tile([P, PW], F32)
                nc.vector.tensor_scalar(out=X[:], in0=xg[:], scalar1=thb[:, 0:1],
                                        scalar2=None, op0=ALU.mult)
                X2 = wp.tile([P, PW], F32)
                nc.vector.scalar_tensor_tensor(out=X2[:], in0=yg[:], scalar=thb[:, 1:2],
                                               in1=X[:], op0=ALU.mult, op1=ALU.add)
                nc.vector.tensor_scalar(out=X[:], in0=X2[:], scalar1=thb[:, 3:4],
                                        scalar2=None, op0=ALU.add)
                Y = wp.tile([P, PW], F32)
                nc.vector.tensor_scalar(out=Y[:], in0=xg[:], scalar1=thb[:, 4:5],
                                        scalar2=None, op0=ALU.mult)
                Y2 = wp.tile([P, PW], F32)
                nc.vector.scalar_tensor_tensor(out=Y2[:], in0=yg[:], scalar=thb[:, 5:6],
                                               in1=Y[:], op0=ALU.mult, op1=ALU.add)
                nc.vector.tensor_scalar(out=Y[:], in0=Y2[:], scalar1=thb[:, 7:8],
                                        scalar2=None, op0=ALU.add)

                # ---- floor via int truncation + negative correction
                def floor_of(src, nm):
                    ti = wp.tile([P, PW], I32, tag="fl_i")
                    nc.vector.tensor_copy(out=ti[:], in_=src[:])
                    tf = wp.tile([P, PW], F32, tag="fl_f")
                    nc.vector.tensor_copy(out=tf[:], in_=ti[:])
                    gt = wp.tile([P, PW], F32, tag="fl_g")
                    nc.vector.tensor_tensor(out=gt[:], in0=tf[:], in1=src[:], op=ALU.is_gt)
                    fl = wp.tile([P, PW], F32, tag=f"fl_o{nm}")
                    nc.vector.tensor_tensor(out=fl[:], in0=tf[:], in1=gt[:], op=ALU.subtract)
                    return fl

                xf = floor_of(X, "x")
                yf = floor_of(Y, "y")

                # ---- fractions, masks, 4 corner weights (wcat order:
                # 0=w00 (y0,x0) 1=w10 (y1,x0) 2=w01 (y0,x1) 3=w11 (y1,x1))
                fx = wp.tile([P, PW], F32)
                nc.vector.tensor_tensor(out=fx[:], in0=X[:], in1=xf[:], op=ALU.subtract)
                fy = wp.tile([P, PW], F32)
                nc.vector.tensor_tensor(out=fy[:], in0=Y[:], in1=yf[:], op=ALU.subtract)
                mgx = wp.tile([P, PW], F32)
                nc.vector.tensor_scalar(out=mgx[:], in0=xf[:], scalar1=0.0, scalar2=None,
                                        op0=ALU.is_ge)
                mx = wp.tile([P, PW], F32)
                nc.vector.scalar_tensor_tensor(out=mx[:], in0=xf[:], scalar=254.0,
                                               in1=mgx[:], op0=ALU.is_le, op1=ALU.mult)
                mgy = wp.tile([P, PW], F32)
                nc.vector.tensor_scalar(out=mgy[:], in0=yf[:], scalar1=0.0, scalar2=None,
                                        op0=ALU.is_ge)
                my = wp.tile([P, PW], F32)
                nc.vector.scalar_tensor_tensor(out=my[:], in0=yf[:], scalar=254.0,
                                               in1=mgy[:], op0=ALU.is_le, op1=ALU.mult)
                fxm = wp.tile([P, PW], F32)
                nc.vector.tensor_tensor(out=fxm[:], in0=fx[:], in1=mx[:], op=ALU.mult)
                alm = wp.tile([P, PW], F32)
                nc.vector.tensor_tensor(out=alm[:], in0=mx[:], in1=fxm[:], op=ALU.subtract)
                fym = wp.tile([P, PW], F32)
                nc.vector.tensor_tensor(out=fym[:], in0=fy[:], in1=my[:], op=ALU.mult)
                gam = wp.tile([P, PW], F32)
                nc.vector.tensor_tensor(out=gam[:], in0=my[:], in1=fym[:], op=ALU.subtract)
                wcat = wp.tile([P, PW, 4], F32)
                nc.vector.tensor_tensor(out=wcat[:, :, 0], in0=alm[:], in1=gam[:], op=ALU.mult)
                nc.vector.tensor_tensor(out=wcat[:, :, 1], in0=alm[:], in1=fym[:], op=ALU.mult)
                nc.vector.tensor_tensor(out=wcat[:, :, 2], in0=fxm[:], in1=gam[:], op=ALU.mult)
                nc.vector.tensor_tensor(out=wcat[:, :, 3], in0=fxm[:], in1=fym[:], op=ALU.mult)

                # ---- sites: xc,yc clamped; mc = xc>>2; s = xc-4mc;
                # site = yc*64+mc
                xc = wp.tile([P, PW], F32)
                nc.vector.tensor_scalar(out=xc[:], in0=xf[:], scalar1=0.0, scalar2=254.0,
                                        op0=ALU.max, op1=ALU.min)
                yc = wp.tile([P, PW], F32)
                nc.vector.tensor_scalar(out=yc[:], in0=yf[:], scalar1=0.0, scalar2=254.0,
                                        op0=ALU.max, op1=ALU.min)
                xq = wp.tile([P, PW], F32, tag="fl_f")
                nc.vector.tensor_scalar(out=xq[:], in0=xc[:], scalar1=0.25, scalar2=None,
                                        op0=ALU.mult)
                mi = wp.tile([P, PW], I32, tag="fl_i")
                nc.vector.tensor_copy(out=mi[:], in_=xq[:])
                mf0 = wp.tile([P, PW], F32, tag="fl_g")
                nc.vector.tensor_copy(out=mf0[:], in_=mi[:])
                gtq = wp.tile([P, PW], F32, tag="mtmp")
                nc.vector.tensor_tensor(out=gtq[:], in0=mf0[:], in1=xq[:], op=ALU.is_gt)
                mf = wp.tile([P, PW], F32)
                nc.vector.tensor_tensor(out=mf[:], in0=mf0[:], in1=gtq[:], op=ALU.subtract)
                sfr = wp.tile([P, PW], F32)
                nc.vector.scalar_tensor_tensor(out=sfr[:], in0=mf[:], scalar=-4.0,
                                               in1=xc[:], op0=ALU.mult, op1=ALU.add)
                lf = wp.tile([P, PW], F32)
                nc.vector.scalar_tensor_tensor(out=lf[:], in0=yc[:], scalar=64.0,
                                               in1=mf[:], op0=ALU.mult, op1=ALU.add)
                off16 = wp.tile([P, PW], I16)
                nc.vector.tensor_copy(out=off16[:], in_=lf[:])

                # ---- e1[p,w,c] = (c == s)  (one-hot within-block position)
                e1 = wp.tile([P, PW, 4], F32)
                i4a = i4[:]
                sfa = sfr[:]
                nc.vector.tensor_tensor(
                    out=e1[:],
                    in0=bass.AP(i4a.tensor, i4a.offset, [list(i4a.ap[0]), [0, PW], [1, 4]]),
                    in1=bass.AP(sfa.tensor, sfa.offset, [list(sfa.ap[0]), [1, PW], [0, 4]]),
                    op=ALU.is_equal)

                # ---- idx marshalling into dma_gather's wrapped layout:
                # idx[16g+e, 8u+t] = site(pixel p=16t+e, w=u)
                idx = xpool.tile([P, 8 * PW], I16)
                if NOSHUF:
                    nc.vector.memset(idx[:], 0)
                else:
                    nc.sync.dma_start(bass.AP(didx, b * P * PW, [[PW, P], [1, PW]]),
                                      off16[:])
                    tmp16 = tpool.tile([P, 8 * PW], I16)
                    for g in range(8):
                        nc.sync.dma_start(
                            tmp16[16 * g:16 * (g + 1), :],
                            bass.AP(didx, b * P * PW, [[PW, 16], [16 * PW, 8], [1, PW]]))
                    tmpa = tmp16[:]
                    idxa = idx[:]
                    nc.vector.tensor_copy(
                        out=bass.AP(idxa.tensor, idxa.offset,
                                    [list(idxa.ap[0]), [1, 8], [8, PW]]),
                        in_=bass.AP(tmpa.tensor, tmpa.offset,
                                    [list(tmpa.ap[0]), [PW, 8], [1, PW]]))

                # ---- gather + extraction + blend, chunked
                ob = opool.tile([P, PW, 3], F32)
                for ci in range(NCHUNK):
                    g64 = gpool.tile([P, CPX, SITE_ELS], F32, tag="g64")
                    if NOGATHER:
                        if b < 2 and ci < 2:
                            nc.vector.memset(g64[:], 0.0)
                    else:
                        nc.gpsimd.dma_gather(
                            out_ap=g64[:, :, :],
                            in_ap=idups[b][:, :],
                            idxs_ap=idx[:, (CPX * 8) * ci:(CPX * 8) * (ci + 1)],
                            num_idxs=CPX * P,
                            num_idxs_reg=CPX * P,
                            elem_size=SITE_ELS,
                            single_packet=False,
                            queue_num=0 if Q0 else ci % NQ,
                        )
                    g64a = g64[:]
                    e1a = e1[:]
                    wca = wcat[:]
                    oba = ob[:]
                    for ch in range(3 if not NOEXT else 0):
                        tmpx = epool.tile([P, CPX * 4, 4], F32, tag="tmpx")
                        txa = tmpx[:]
                        nc.vector.tensor_tensor(
                            out=bass.AP(txa.tensor, txa.offset,
                                        [list(txa.ap[0]), [16, CPX], [4, 4], [1, 4]]),
                            in0=bass.AP(e1a.tensor, e1a.offset + 4 * CPX * ci,
                                        [list(e1a.ap[0]), [4, CPX], [0, 4], [1, 4]]),
                            in1=bass.AP(g64a.tensor, g64a.offset + ch,
                                        [list(g64a.ap[0]), [SITE_ELS, CPX], [3, 4], [6, 4]]),
                            op=ALU.mult)
                        x3 = epool.tile([P, CPX * 4], F32, tag="x3")
                        nc.vector.tensor_reduce(out=x3[:], in_=tmpx[:], axis=AX.X,
                                                op=ALU.add)
                        t2 = epool.tile([P, CPX, 4], F32, tag="t2")
                        x3a = x3[:]
                        nc.vector.tensor_tensor(
                            out=t2[:],
                            in0=bass.AP(x3a.tensor, x3a.offset,
                                        [list(x3a.ap[0]), [4, CPX], [1, 4]]),
                            in1=bass.AP(wca.tensor, wca.offset + 4 * CPX * ci,
                                        [list(wca.ap[0]), [4, CPX], [1, 4]]),
                            op=ALU.mult)
                        nc.vector.tensor_reduce(
                            out=bass.AP(oba.tensor, oba.offset + 3 * CPX * ci + ch,
                                        [list(oba.ap[0]), [3, CPX], [1, 1]]),
                            in_=t2[:], axis=AX.X, op=ALU.add)

                nc.sync.dma_start(bass.AP(out_d, b * IMG_ELS, [[PW * 3, P], [1, PW * 3]]),
                                  ob[:])
    nc.compile()
    return nc


def _build_i1(n_imgs):
    """Original baseline: per-column [P,1] indirect DMA gathers."""
    nc = bacc.Bacc("TRN2", target_bir_lowering=False, debug=False,
                   enable_asserts=False, num_devices=1, num_swdge_queues=2)
    inp = nc.dram_tensor("inp", [n_imgs, 6 + IMG_ELS], F32, kind="ExternalInput")
    xg_d = nc.dram_tensor("xg", [P, PW], F32, kind="ExternalInput")
    yg_d = nc.dram_tensor("yg", [P, PW], F32, kind="ExternalInput")
    cst_d = nc.dram_tensor("cst", [2, 4], F32, kind="ExternalInput")
    i4_d = nc.dram_tensor("i4", [P, 4], F32, kind="ExternalInput")  # unused
    out_d = nc.dram_tensor("out", [n_imgs, H * W * C], F32, kind="ExternalOutput")
    idups = [nc.dram_tensor(f"idup{b}", [H * W, 6], F32) for b in range(n_imgs)]
    scr = nc.dram_tensor("scr", [n_imgs, 8], F32)

    with tile.TileContext(nc) as tc:
        with (
            tc.tile_pool(name="const", bufs=1) as cpool,
            tc.tile_pool(name="work", bufs=1) as wp,
            tc.tile_pool(name="gath", bufs=2) as gpool,
            tc.tile_pool(name="offp", bufs=2) as opool,
        ):
            xg = cpool.tile([P, PW], F32)
            nc.sync.dma_start(xg[:], xg_d[:, :])
            yg = cpool.tile([P, PW], F32)
            nc.sync.dma_start(yg[:], yg_d[:, :])
            cst = cpool.tile([2, 4], F32)
            nc.sync.dma_start(cst[:], cst_d[:, :])

            for b in range(n_imgs):
                th = wp.tile([2, 3], F32)
                nc.sync.dma_start(th[:], bass.AP(inp, b * (6 + IMG_ELS), [[3, 2], [1, 3]]))
                m = wp.tile([2, 3], F32)
                nc.vector.tensor_tensor(out=m[:], in0=th[:], in1=cst[:, 0:3], op=ALU.mult)
                s = wp.tile([2, 1], F32)
                nc.vector.tensor_reduce(out=s[:], in_=m[:], axis=AX.X, op=ALU.add)
                pr = wp.tile([2, 4], F32)
                nc.vector.tensor_copy(out=pr[:, 0:3], in_=th[:])
                nc.vector.tensor_scalar(out=pr[:, 3:4], in0=s[:], scalar1=127.5,
                                        scalar2=None, op0=ALU.add)
                nc.sync.dma_start(bass.AP(scr, b * 8, [[4, 2], [1, 4]]), pr[:])
                thb = wp.tile([P, 8], F32)
                nc.sync.dma_start(thb[:], bass.AP(scr, b * 8, [[0, P], [1, 8]]))

                it = wp.tile([P, 1536], F32)
                nc.sync.dma_start(it[:], bass.AP(inp, b * (6 + IMG_ELS) + 6,
                                                 [[1536, P], [1, 1536]]))
                hal = wp.tile([P, ROW_ELS], F32)
                nc.sync.dma_start(hal[0:127, :],
                                  bass.AP(inp, b * (6 + IMG_ELS) + 6 + 1536,
                                          [[1536, 127], [1, ROW_ELS]]))
                nc.sync.dma_start(hal[127:128, :],
                                  bass.AP(inp, b * (6 + IMG_ELS) + 6 + IMG_ELS - ROW_ELS,
                                          [[ROW_ELS, 1], [1, ROW_ELS]]))
                d2 = wp.tile([P, PW, 6], F32)
                it3 = it[:].rearrange("p (w c) -> p w c", c=3)
                nc.vector.tensor_copy(out=d2[:, :, 0:3], in_=it3)
                nc.vector.tensor_copy(out=d2[:, 0:256, 3:6],
                                      in_=it[:, ROW_ELS:1536].rearrange("p (w c) -> p w c", c=3))
                nc.vector.tensor_copy(out=d2[:, 256:512, 3:6],
                                      in_=hal[:].rearrange("p (w c) -> p w c", c=3))
                nc.sync.dma_start(idups[b][:, :], d2[:])

                X = wp.tile([P, PW], F32)
                nc.vector.tensor_scalar(out=X[:], in0=xg[:], scalar1=thb[:, 0:1],
                                        scalar2=None, op0=ALU.mult)
                X2 = wp.tile([P, PW], F32)
                nc.vector.scalar_tensor_tensor(out=X2[:], in0=yg[:], scalar=thb[:, 1:2],
                                               in1=X[:], op0=ALU.mult, op1=ALU.add)
                nc.vector.tensor_scalar(out=X[:], in0=X2[:], scalar1=thb[:, 3:4],
                                        scalar2=None, op0=ALU.add)
                Y = wp.tile([P, PW], F32)
                nc.vector.tensor_scalar(out=Y[:], in0=xg[:], scalar1=thb[:, 4:5],
                                        scalar2=None, op0=ALU.mult)
                Y2 = wp.tile([P, PW], F32)
                nc.vector.scalar_tensor_tensor(out=Y2[:], in0=yg[:], scalar=thb[:, 5:6],
                                               in1=Y[:], op0=ALU.mult, op1=ALU.add)
                nc.vector.tensor_scalar(out=Y[:], in0=Y2[:], scalar1=thb[:, 7:8],
                                        scalar2=None, op0=ALU.add)

                def floor_of(src, nm):
                    ti = wp.tile([P, PW], I32, tag=f"fl_i{nm}")
                    nc.vector.tensor_copy(out=ti[:], in_=src[:])
                    tf = wp.tile([P, PW], F32, tag=f"fl_f{nm}")
                    nc.vector.tensor_copy(out=tf[:], in_=ti[:])
                    gt = wp.tile([P, PW], F32, tag=f"fl_g{nm}")
                    nc.vector.tensor_tensor(out=gt[:], in0=tf[:], in1=src[:], op=ALU.is_gt)
                    fl = wp.tile([P, PW], F32, tag=f"fl_o{nm}")
                    nc.vector.tensor_tensor(out=fl[:], in0=tf[:], in1=gt[:], op=ALU.subtract)
                    return fl

                xf = floor_of(X, "x")
                yf = floor_of(Y, "y")

                fx = wp.tile([P, PW], F32)
                nc.vector.tensor_tensor(out=fx[:], in0=X[:], in1=xf[:], op=ALU.subtract)
                fy = wp.tile([P, PW], F32)
                nc.vector.tensor_tensor(out=fy[:], in0=Y[:], in1=yf[:], op=ALU.subtract)
                al = wp.tile([P, PW], F32)
                nc.vector.tensor_scalar(out=al[:], in0=fx[:], scalar1=-1.0, scalar2=1.0,
                                        op0=ALU.mult, op1=ALU.add)
                ga = wp.tile([P, PW], F32)
                nc.vector.tensor_scalar(out=ga[:], in0=fy[:], scalar1=-1.0, scalar2=1.0,
                                        op0=ALU.mult, op1=ALU.add)
                mgx = wp.tile([P, PW], F32)
                nc.vector.tensor_scalar(out=mgx[:], in0=xf[:], scalar1=0.0, scalar2=None,
                                        op0=ALU.is_ge)
                mx = wp.tile([P, PW], F32)
                nc.vector.scalar_tensor_tensor(out=mx[:], in0=xf[:], scalar=254.0,
                                               in1=mgx[:], op0=ALU.is_le, op1=ALU.mult)
                mgy = wp.tile([P, PW], F32)
                nc.vector.tensor_scalar(out=mgy[:], in0=yf[:], scalar1=0.0, scalar2=None,
                                        op0=ALU.is_ge)
                my = wp.tile([P, PW], F32)
                nc.vector.scalar_tensor_tensor(out=my[:], in0=yf[:], scalar=254.0,
                                               in1=mgy[:], op0=ALU.is_le, op1=ALU.mult)
                A = wp.tile([P, PW], F32)
                nc.vector.tensor_tensor(out=A[:], in0=al[:], in1=mx[:], op=ALU.mult)
                Bw = wp.tile([P, PW], F32)
                nc.vector.tensor_tensor(out=Bw[:], in0=fx[:], in1=mx[:], op=ALU.mult)
                Cw = wp.tile([P, PW], F32)
                nc.vector.tensor_tensor(out=Cw[:], in0=ga[:], in1=my[:], op=ALU.mult)
                Dw = wp.tile([P, PW], F32)
                nc.vector.tensor_tensor(out=Dw[:], in0=fy[:], in1=my[:], op=ALU.mult)
                w00 = wp.tile([P, PW], F32)
                nc.vector.tensor_tensor(out=w00[:], in0=Cw[:], in1=A[:], op=ALU.mult)
                w10 = wp.tile([P, PW], F32)
                nc.vector.tensor_tensor(out=w10[:], in0=Dw[:], in1=A[:], op=ALU.mult)
                w01 = wp.tile([P, PW], F32)
                nc.vector.tensor_tensor(out=w01[:], in0=Cw[:], in1=Bw[:], op=ALU.mult)
                w11 = wp.tile([P, PW], F32)
                nc.vector.tensor_tensor(out=w11[:], in0=Dw[:], in1=Bw[:], op=ALU.mult)

                xc = wp.tile([P, PW], F32)
                nc.vector.tensor_scalar(out=xc[:], in0=xf[:], scalar1=0.0, scalar2=254.0,
                                        op0=ALU.max, op1=ALU.min)
                yc = wp.tile([P, PW], F32)
                nc.vector.tensor_scalar(out=yc[:], in0=yf[:], scalar1=0.0, scalar2=254.0,
                                        op0=ALU.max, op1=ALU.min)
                lf = wp.tile([P, PW], F32)
                nc.vector.scalar_tensor_tensor(out=lf[:], in0=yc[:], scalar=256.0,
                                               in1=xc[:], op0=ALU.mult, op1=ALU.add)
                off = opool.tile([P, PW], I32)
                nc.vector.tensor_copy(out=off[:], in_=lf[:])

                g = gpool.tile([P, PW, 12], F32)
                for w in range(PW):
                    inst = nc.gpsimd.indirect_dma_start(
                        out=g[:, w, :], out_offset=None,
                        in_=idups[b][:, :],
                        in_offset=bass.IndirectOffsetOnAxis(ap=off[:, w:w + 1], axis=0))
                    if w % 2:
                        inst.ins.queue = "qPoolDynamic1"

                def bc3(t):
                    return bass.AP(t.tensor, t.offset, list(t.ap) + [[0, 3]])

                t0 = wp.tile([P, PW, 3], F32)
                nc.vector.tensor_tensor(out=t0[:], in0=g[:, :, 0:3], in1=bc3(w00[:]), op=ALU.mult)
                t1 = wp.tile([P, PW, 3], F32)
                nc.vector.tensor_tensor(out=t1[:], in0=g[:, :, 3:6], in1=bc3(w10[:]), op=ALU.mult)
                t2 = wp.tile([P, PW, 3], F32)
                nc.vector.tensor_tensor(out=t2[:], in0=g[:, :, 6:9], in1=bc3(w01[:]), op=ALU.mult)
                t3 = wp.tile([P, PW, 3], F32)
                nc.vector.tensor_tensor(out=t3[:], in0=g[:, :, 9:12], in1=bc3(w11[:]), op=ALU.mult)
                nc.vector.tensor_tensor(out=t0[:], in0=t0[:], in1=t1[:], op=ALU.add)
                nc.vector.tensor_tensor(out=t2[:], in0=t2[:], in1=t3[:], op=ALU.add)
                ob = wp.tile([P, PW, 3], F32)
                nc.vector.tensor_tensor(out=ob[:], in0=t0[:], in1=t2[:], op=ALU.add)
                nc.sync.dma_start(bass.AP(out_d, b * IMG_ELS, [[PW * 3, P], [1, PW * 3]]),
                                  ob[:])
    nc.compile()
    return nc


def _build(n_imgs):
    if GMODE == "dg":
        return _build_dg(n_imgs)
    return _build_i1(n_imgs)


def _consts():
    # pixel (p, w): global l = p*PW + w ; j = l % 256 ; i = l // 256
    l = np.arange(P * PW).reshape(P, PW)
    xg = (l % 256).astype(np.float32)
    yg = (l // 256).astype(np.float32)
    cst = np.tile(np.array([-127.5, -127.5, 127.5, 0.0], np.float32), (2, 1))
    i4 = np.tile(np.arange(4, dtype=np.float32), (P, 1))
    return xg, yg, cst, i4


IMGS_PER_LAUNCH = 16


def kernel(inputs: np.ndarray) -> np.ndarray:
    inputs = np.ascontiguousarray(inputs, dtype=np.float32)
    assert inputs.shape == (128, 6 + IMG_ELS)
    npl = IMGS_PER_LAUNCH
    key = (npl, GMODE)
    if key not in _cached:
        _cached[key] = _build(npl)
    nc = _cached[key]
    xg, yg, cst, i4 = _consts()
    trace = bool(os.environ.get("BILIN_TRACE"))
    if trace:
        try:  # NTFF trace hook is missing from this image's antenv; install shim
            import antenv.axon_hooks  # noqa: F401
        except ImportError:
            try:
                import types
                from trn_agent_boot.trn_boot import _ntff_profile_via_ctypes
                hook = _ntff_profile_via_ctypes("/opt/axon/libaxon_pjrt.so")
                mod = types.ModuleType("antenv.axon_hooks")
                mod.get_axon_ntff_profile_hook = lambda: hook
                sys.modules["antenv.axon_hooks"] = mod
            except Exception:
                trace = False
    out = np.empty((128, H, W, C), np.float32)
    total_ns = 0
    n_launches = IMGS // npl
    for k in range(n_launches):
        in_maps = []
        for c in range(N_CORES):
            lo = c * IMGS + k * npl
            in_maps.append(dict(inp=np.ascontiguousarray(inputs[lo:lo + npl]),
                                xg=xg, yg=yg, cst=cst, i4=i4))
        res = run_bass_kernel_spmd(nc, in_maps, core_ids=list(range(N_CORES)),
                                   trace=trace and k == 0)
        if trace and k == 0 and res.exec_time_ns is not None:
            total_ns = res.exec_time_ns * n_launches
        for c in range(N_CORES):
            lo = c * IMGS + k * npl
            out[lo:lo + npl] = res.results[c]["out"].reshape(npl, H, W, C)
    if trace:
        print(f"HW exec time: {total_ns} ns")
    return out


# revision 14
# speedup vs baseline: 1.7944x; 1.2630x over previous
"""Bilinear sampler (spatial transformer) TRN2 Bass kernel.

Contract: kernel(inputs=[128, 196614] fp32) -> [128, 256, 256, 3] fp32.
Shards batch over 8 NeuronCores (16 images each).

Mode 'dg' (default): per image, build a 256B-aligned site table in DRAM
(site = y0*64 + x0//4 holds rows y0,y0+1 x cols 4m..4m+4, 30 used floats
padded to 64), then fetch one 256B block per output pixel with
InstDMAGatherAnt (vectorized int16 indices, ~0.34ns/desc gen vs ~1.3us
per 128-desc INDIRECT1D instruction).  The variable within-block
position (s = x0 mod 4) is resolved by a one-hot mask e1[s] contracted
against the 4 candidate patch windows during the blend.

Mode 'i1': original per-column [P,1] indirect-DMA gather baseline.
"""
import os
import sys

sys.path.insert(0, "/opt/trn_rl_repo")

import numpy as np

import concourse.bacc as bacc
import concourse.bass as bass
import concourse.mybir as mybir
import concourse.tile as tile
from concourse.bass_utils import run_bass_kernel_spmd

P = 128
H = W = 256
C = 3
IMG_ELS = H * W * C            # 196608
ROW_ELS = W * C                # 768
PW = (H * W) // P              # 512 pixels per partition per image
N_CORES = 8
IMGS = 16                      # images per core

F32 = mybir.dt.float32
I32 = mybir.dt.int32
I16 = mybir.dt.int16
ALU = mybir.AluOpType
AX = mybir.AxisListType

GMODE = os.environ.get("BILIN_GMODE", "dg")
NOGATHER = bool(os.environ.get("BILIN_NOGATHER"))
NOSHUF = bool(os.environ.get("BILIN_NOSHUF"))
NOEXT = bool(os.environ.get("BILIN_NOEXT"))
Q0 = bool(os.environ.get("BILIN_Q0"))
NQ = int(os.environ.get("BILIN_NQ", "2"))
NCHUNK = int(os.environ.get("BILIN_NCHUNK", "8"))  # extraction groups/image
GPC = int(os.environ.get("BILIN_GPC", "8"))  # w-cols per dma_gather (8 -> 1024
# idxs = 64 descriptors = one packet: single-packet CounterMachine fast path)
CPX = PW // NCHUNK             # 64 w-columns per chunk
NSITE = 16384                  # sites per image: 256 rowpairs x 64 col-blocks
SITE_ELS = 64                  # 256B per site

_cached = {}


def _ap(t, extra_off, dims):
    """Strided view of a tile AP: keep partition dim, custom free dims."""
    return bass.AP(t.tensor, t.offset + extra_off, [list(t.ap[0])] + dims)


def _build_dg(n_imgs):
    nc = bacc.Bacc("TRN2", target_bir_lowering=False, debug=False,
                   enable_asserts=False, num_devices=1, num_swdge_queues=NQ)
    inp = nc.dram_tensor("inp", [n_imgs, 6 + IMG_ELS], F32, kind="ExternalInput")
    xg_d = nc.dram_tensor("xg", [P, PW], F32, kind="ExternalInput")
    yg_d = nc.dram_tensor("yg", [P, PW], F32, kind="ExternalInput")
    cst_d = nc.dram_tensor("cst", [2, 4], F32, kind="ExternalInput")
    i4_d = nc.dram_tensor("i4", [P, 4], F32, kind="ExternalInput")
    out_d = nc.dram_tensor("out", [n_imgs, IMG_ELS], F32, kind="ExternalOutput")
    idups = [nc.dram_tensor(f"idup{b}", [NSITE, SITE_ELS], F32)
             for b in range(n_imgs)]
    didx = nc.dram_tensor("didx", [n_imgs, P * PW], I16)
    scr = nc.dram_tensor("scr", [n_imgs, 8], F32)

    with tile.TileContext(nc) as tc:
        with (
            tc.tile_pool(name="const", bufs=1) as cpool,
            tc.tile_pool(name="img", bufs=2) as ipool,
            tc.tile_pool(name="d2p", bufs=1) as d2pool,
            tc.tile_pool(name="work", bufs=1) as wp,
            tc.tile_pool(name="gath", bufs=2) as gpool,
            tc.tile_pool(name="tmpp", bufs=1) as tpool,
            tc.tile_pool(name="idxp", bufs=2) as xpool,
            tc.tile_pool(name="ext", bufs=2) as epool,
            tc.tile_pool(name="outp", bufs=2) as opool,
        ):
            xg = cpool.tile([P, PW], F32)
            nc.sync.dma_start(xg[:], xg_d[:, :])
            yg = cpool.tile([P, PW], F32)
            nc.sync.dma_start(yg[:], yg_d[:, :])
            cst = cpool.tile([2, 4], F32)
            nc.sync.dma_start(cst[:], cst_d[:, :])
            i4 = cpool.tile([P, 4], F32)
            nc.sync.dma_start(i4[:], i4_d[:, :])

            for b in range(n_imgs):
                # ---- affine params: thb cols 0=t00 1=t01 3=cx 4=t10 5=t11 7=cy
                th = wp.tile([2, 3], F32)
                nc.sync.dma_start(th[:], bass.AP(inp, b * (6 + IMG_ELS), [[3, 2], [1, 3]]))
                m = wp.tile([2, 3], F32)
                nc.vector.tensor_tensor(out=m[:], in0=th[:], in1=cst[:, 0:3], op=ALU.mult)
                s = wp.tile([2, 1], F32)
                nc.vector.tensor_reduce(out=s[:], in_=m[:], axis=AX.X, op=ALU.add)
                pr = wp.tile([2, 4], F32)
                nc.vector.tensor_copy(out=pr[:, 0:3], in_=th[:])
                nc.vector.tensor_scalar(out=pr[:, 3:4], in0=s[:], scalar1=127.5,
                                        scalar2=None, op0=ALU.add)
                nc.sync.dma_start(bass.AP(scr, b * 8, [[4, 2], [1, 4]]), pr[:])
                thb = wp.tile([P, 8], F32)
                nc.sync.dma_start(thb[:], bass.AP(scr, b * 8, [[0, P], [1, 8]]))

                # ---- image loads: it rows 2p,2p+1; hal row 2p+2 (padded)
                it = ipool.tile([P, 1544], F32)
                hal = ipool.tile([P, 776], F32)
                if b < 2:
                    nc.vector.memset(it[:], 0.0)
                    nc.vector.memset(hal[:], 0.0)
                nc.sync.dma_start(it[:, 0:1536],
                                  bass.AP(inp, b * (6 + IMG_ELS) + 6,
                                          [[1536, P], [1, 1536]]))
                nc.sync.dma_start(hal[0:127, 0:ROW_ELS],
                                  bass.AP(inp, b * (6 + IMG_ELS) + 6 + 1536,
                                          [[1536, 127], [1, ROW_ELS]]))
                nc.sync.dma_start(hal[127:128, 0:ROW_ELS],
                                  bass.AP(inp, b * (6 + IMG_ELS) + 6 + IMG_ELS - ROW_ELS,
                                          [[ROW_ELS, 1], [1, ROW_ELS]]))

                # ---- d2: [2 rp, 64 m, 64] per partition; site (2p+rp)*64+m
                # holds offset 6c+3sr+ch = row (2p+rp+sr), col (4m+c), c<5
                d2 = d2pool.tile([P, 2 * 64 * SITE_ELS], F32)
                if b < 2:
                    nc.vector.memset(d2[:], 0.0)
                d2a = d2[:]
                ita = it[:]
                hala = hal[:]
                for rp in range(2):
                    for sr in range(2):
                        row = rp + sr           # 0,1,2 relative to 2p
                        if row < 2:
                            src = _ap(ita, row * ROW_ELS, [[12, 64], [3, 5], [1, 3]])
                        else:
                            src = _ap(hala, 0, [[12, 64], [3, 5], [1, 3]])
                        dst = _ap(d2a, rp * 64 * SITE_ELS + 3 * sr,
                                  [[SITE_ELS, 64], [6, 5], [1, 3]])
                        nc.vector.tensor_copy(out=dst, in_=src)
                nc.sync.dma_start(idups[b][:, :], d2[:])

                # ---- grid coords (pixel units)
                X = wp.tile([P, PW], F32)
                nc.vector.tensor_scalar(out=X[:], in0=xg[:], scalar1=thb[:, 0:1],
                                        scalar2=None, op0=ALU.mult)
                X2 = wp.tile([P, PW], F32)
                nc.vector.scalar_tensor_tensor(out=X2[:], in0=yg[:], scalar=thb[:, 1:2],
                                               in1=X[:], op0=ALU.mult, op1=ALU.add)
                nc.vector.tensor_scalar(out=X[:], in0=X2[:], scalar1=thb[:, 3:4],
                                        scalar2=None, op0=ALU.add)
                Y = wp.tile([P, PW], F32)
                nc.vector.tensor_scalar(out=Y[:], in0=xg[:], scalar1=thb[:, 4:5],
                                        scalar2=None, op0=ALU.mult)
                Y2 = wp.tile([P, PW], F32)
                nc.vector.scalar_tensor_tensor(out=Y2[:], in0=yg[:], scalar=thb[:, 5:6],
                                               in1=Y[:], op0=ALU.mult, op1=ALU.add)
                nc.vector.tensor_scalar(out=Y[:], in0=Y2[:], scalar1=thb[:, 7:8],
                                        scalar2=None, op0=ALU.add)

                # ---- floor via int truncation + negative correction
                def floor_of(src, nm):
                    ti = wp.tile([P, PW], I32, tag="fl_i")
                    nc.vector.tensor_copy(out=ti[:], in_=src[:])
                    tf = wp.tile([P, PW], F32, tag="fl_f")
                    nc.vector.tensor_copy(out=tf[:], in_=ti[:])
                    gt = wp.tile([P, PW], F32, tag="fl_g")
                    nc.vector.tensor_tensor(out=gt[:], in0=tf[:], in1=src[:], op=ALU.is_gt)
                    fl = wp.tile([P, PW], F32, tag=f"fl_o{nm}")
                    nc.vector.tensor_tensor(out=fl[:], in0=tf[:], in1=gt[:], op=ALU.subtract)
                    return fl

                xf = floor_of(X, "x")
                yf = floor_of(Y, "y")

                # ---- fractions, masks, 4 corner weights (wcat order:
                # 0=w00 (y0,x0) 1=w10 (y1,x0) 2=w01 (y0,x1) 3=w11 (y1,x1))
                fx = wp.tile([P, PW], F32)
                nc.vector.tensor_tensor(out=fx[:], in0=X[:], in1=xf[:], op=ALU.subtract)
                fy = wp.tile([P, PW], F32)
                nc.vector.tensor_tensor(out=fy[:], in0=Y[:], in1=yf[:], op=ALU.subtract)
                mgx = wp.tile([P, PW], F32)
                nc.vector.tensor_scalar(out=mgx[:], in0=xf[:], scalar1=0.0, scalar2=None,
                                        op0=ALU.is_ge)
                mx = wp.tile([P, PW], F32)
                nc.vector.scalar_tensor_tensor(out=mx[:], in0=xf[:], scalar=254.0,
                                               in1=mgx[:], op0=ALU.is_le, op1=ALU.mult)
                mgy = wp.tile([P, PW], F32)
                nc.vector.tensor_scalar(out=mgy[:], in0=yf[:], scalar1=0.0, scalar2=None,
                                        op0=ALU.is_ge)
                my = wp.tile([P, PW], F32)
                nc.vector.scalar_tensor_tensor(out=my[:], in0=yf[:], scalar=254.0,
                                               in1=mgy[:], op0=ALU.is_le, op1=ALU.mult)
                fxm = wp.tile([P, PW], F32)
                nc.vector.tensor_tensor(out=fxm[:], in0=fx[:], in1=mx[:], op=ALU.mult)
                alm = wp.tile([P, PW], F32)
                nc.vector.tensor_tensor(out=alm[:], in0=mx[:], in1=fxm[:], op=ALU.subtract)
                fym = wp.tile([P, PW], F32)
                nc.vector.tensor_tensor(out=fym[:], in0=fy[:], in1=my[:], op=ALU.mult)
                gam = wp.tile([P, PW], F32)
                nc.vector.tensor_tensor(out=gam[:], in0=my[:], in1=fym[:], op=ALU.subtract)
                wcat = wp.tile([P, PW, 4], F32)
                nc.vector.tensor_tensor(out=wcat[:, :, 0], in0=alm[:], in1=gam[:], op=ALU.mult)
                nc.vector.tensor_tensor(out=wcat[:, :, 1], in0=alm[:], in1=fym[:], op=ALU.mult)
                nc.vector.tensor_tensor(out=wcat[:, :, 2], in0=fxm[:], in1=gam[:], op=ALU.mult)
                nc.vector.tensor_tensor(out=wcat[:, :, 3], in0=fxm[:], in1=fym[:], op=ALU.mult)

                # ---- sites: xc,yc clamped; mc = xc>>2; s = xc-4mc;
                # site = yc*64+mc
                xc = wp.tile([P, PW], F32)
                nc.vector.tensor_scalar(out=xc[:], in0=xf[:], scalar1=0.0, scalar2=254.0,
                                        op0=ALU.max, op1=ALU.min)
                yc = wp.tile([P, PW], F32)
                nc.vector.tensor_scalar(out=yc[:], in0=yf[:], scalar1=0.0, scalar2=254.0,
                                        op0=ALU.max, op1=ALU.min)
                xq = wp.tile([P, PW], F32, tag="fl_f")
                nc.vector.tensor_scalar(out=xq[:], in0=xc[:], scalar1=0.25, scalar2=None,
                                        op0=ALU.mult)
                mi = wp.tile([P, PW], I32, tag="fl_i")
                nc.vector.tensor_copy(out=mi[:], in_=xq[:])
                mf0 = wp.tile([P, PW], F32, tag="fl_g")
                nc.vector.tensor_copy(out=mf0[:], in_=mi[:])
                gtq = wp.tile([P, PW], F32, tag="mtmp")
                nc.vector.tensor_tensor(out=gtq[:], in0=mf0[:], in1=xq[:], op=ALU.is_gt)
                mf = wp.tile([P, PW], F32)
                nc.vector.tensor_tensor(out=mf[:], in0=mf0[:], in1=gtq[:], op=ALU.subtract)
                sfr = wp.tile([P, PW], F32)
                nc.vector.scalar_tensor_tensor(out=sfr[:], in0=mf[:], scalar=-4.0,
                                               in1=xc[:], op0=ALU.mult, op1=ALU.add)
                lf = wp.tile([P, PW], F32)
                nc.vector.scalar_tensor_tensor(out=lf[:], in0=yc[:], scalar=64.0,
                                               in1=mf[:], op0=ALU.mult, op1=ALU.add)
                off16 = wp.tile([P, PW], I16)
                nc.vector.tensor_copy(out=off16[:], in_=lf[:])

                # ---- e1[p,w,c] = (c == s)  (one-hot within-block position)
                e1 = wp.tile([P, PW, 4], F32)
                i4a = i4[:]
                sfa = sfr[:]
                nc.vector.tensor_tensor(
                    out=e1[:],
                    in0=bass.AP(i4a.tensor, i4a.offset, [list(i4a.ap[0]), [0, PW], [1, 4]]),
                    in1=bass.AP(sfa.tensor, sfa.offset, [list(sfa.ap[0]), [1, PW], [0, 4]]),
                    op=ALU.is_equal)

                # ---- idx marshalling into dma_gather's wrapped layout:
                # idx[16g+e, 8u+t] = site(pixel p=16t+e, w=u)
                idx = xpool.tile([P, 8 * PW], I16)
                if NOSHUF:
                    nc.vector.memset(idx[:], 0)
                else:
                    nc.sync.dma_start(bass.AP(didx, b * P * PW, [[PW, P], [1, PW]]),
                                      off16[:])
                    tmp16 = tpool.tile([P, 8 * PW], I16)
                    for g in range(8):
                        nc.sync.dma_start(
                            tmp16[16 * g:16 * (g + 1), :],
                            bass.AP(didx, b * P * PW, [[PW, 16], [16 * PW, 8], [1, PW]]))
                    tmpa = tmp16[:]
                    idxa = idx[:]
                    nc.vector.tensor_copy(
                        out=bass.AP(idxa.tensor, idxa.offset,
                                    [list(idxa.ap[0]), [1, 8], [8, PW]]),
                        in_=bass.AP(tmpa.tensor, tmpa.offset,
                                    [list(tmpa.ap[0]), [PW, 8], [1, PW]]))

                # ---- gather + extraction + blend
                # GPC-column gathers (single-packet fast path needs
                # <=64 descriptors = 1024 idxs) fill an EXT-column buffer;
                # extraction runs once per buffer to keep DVE ops big.
                ob = opool.tile([P, PW, 3], F32)
                for ci in range(NCHUNK):
                    g64 = gpool.tile([P, CPX, SITE_ELS], F32, tag="g64")
                    if NOGATHER:
                        if b < 2 and ci < 2:
                            nc.vector.memset(g64[:], 0.0)
                    else:
                        for gi in range(CPX // GPC):
                            lo = ci * CPX + gi * GPC
                            nc.gpsimd.dma_gather(
                                out_ap=g64[:, gi * GPC:(gi + 1) * GPC, :],
                                in_ap=idups[b][:, :],
                                idxs_ap=idx[:, GPC * 8 * (lo // GPC):
                                            GPC * 8 * (lo // GPC + 1)],
                                num_idxs=GPC * P,
                                num_idxs_reg=GPC * P,
                                elem_size=SITE_ELS,
                                single_packet=(GPC * P <= 1024),
                                queue_num=0 if Q0 else (lo // GPC) % NQ,
                            )
                    g64a = g64[:]
                    e1a = e1[:]
                    wca = wcat[:]
                    oba = ob[:]
                    for ch in range(3 if not NOEXT else 0):
                        tmpx = epool.tile([P, CPX * 4, 4], F32, tag="tmpx")
                        txa = tmpx[:]
                        nc.vector.tensor_tensor(
                            out=bass.AP(txa.tensor, txa.offset,
                                        [list(txa.ap[0]), [16, CPX], [4, 4], [1, 4]]),
                            in0=bass.AP(e1a.tensor, e1a.offset + 4 * CPX * ci,
                                        [list(e1a.ap[0]), [4, CPX], [0, 4], [1, 4]]),
                            in1=bass.AP(g64a.tensor, g64a.offset + ch,
                                        [list(g64a.ap[0]), [SITE_ELS, CPX], [3, 4], [6, 4]]),
                            op=ALU.mult)
                        x3 = epool.tile([P, CPX * 4], F32, tag="x3")
                        nc.vector.tensor_reduce(out=x3[:], in_=tmpx[:], axis=AX.X,
                                                op=ALU.add)
                        t2 = epool.tile([P, CPX, 4], F32, tag="t2")
                        x3a = x3[:]
                        nc.vector.tensor_tensor(
                            out=t2[:],
                            in0=bass.AP(x3a.tensor, x3a.offset,
                                        [list(x3a.ap[0]), [4, CPX], [1, 4]]),
                            in1=bass.AP(wca.tensor, wca.offset + 4 * CPX * ci,
                                        [list(wca.ap[0]), [4, CPX], [1, 4]]),
                            op=ALU.mult)
                        nc.vector.tensor_reduce(
                            out=bass.AP(oba.tensor, oba.offset + 3 * CPX * ci + ch,
                                        [list(oba.ap[0]), [3, CPX], [1, 1]]),
                            in_=t2[:], axis=AX.X, op=ALU.add)

                nc.sync.dma_start(bass.AP(out_d, b * IMG_ELS, [[PW * 3, P], [1, PW * 3]]),
                                  ob[:])
    nc.compile()
    return nc


def _build_i1(n_imgs):
    """Original baseline: per-column [P,1] indirect DMA gathers."""
    nc = bacc.Bacc("TRN2", target_bir_lowering=False, debug=False,
                   enable_asserts=False, num_devices=1, num_swdge_queues=2)
    inp = nc.dram_tensor("inp", [n_imgs, 6 + IMG_ELS], F32, kind="ExternalInput")
    xg_d = nc.dram_tensor("xg", [P, PW], F32, kind="ExternalInput")
    yg_d = nc.dram_tensor("yg", [P, PW], F32, kind="ExternalInput")
    cst_d = nc.dram_tensor("cst", [2, 4], F32, kind="ExternalInput")
    i4_d = nc.dram_tensor("i4", [P, 4], F32, kind="ExternalInput")  # unused
    out_d = nc.dram_tensor("out", [n_imgs, H * W * C], F32, kind="ExternalOutput")
    idups = [nc.dram_tensor(f"idup{b}", [H * W, 6], F32) for b in range(n_imgs)]
    scr = nc.dram_tensor("scr", [n_imgs, 8], F32)

    with tile.TileContext(nc) as tc:
        with (
            tc.tile_pool(name="const", bufs=1) as cpool,
            tc.tile_pool(name="work", bufs=1) as wp,
            tc.tile_pool(name="gath", bufs=2) as gpool,
            tc.tile_pool(name="offp", bufs=2) as opool,
        ):
            xg = cpool.tile([P, PW], F32)
            nc.sync.dma_start(xg[:], xg_d[:, :])
            yg = cpool.tile([P, PW], F32)
            nc.sync.dma_start(yg[:], yg_d[:, :])
            cst = cpool.tile([2, 4], F32)
            nc.sync.dma_start(cst[:], cst_d[:, :])

            for b in range(n_imgs):
                th = wp.tile([2, 3], F32)
                nc.sync.dma_start(th[:], bass.AP(inp, b * (6 + IMG_ELS), [[3, 2], [1, 3]]))
                m = wp.tile([2, 3], F32)
                nc.vector.tensor_tensor(out=m[:], in0=th[:], in1=cst[:, 0:3], op=ALU.mult)
                s = wp.tile([2, 1], F32)
                nc.vector.tensor_reduce(out=s[:], in_=m[:], axis=AX.X, op=ALU.add)
                pr = wp.tile([2, 4], F32)
                nc.vector.tensor_copy(out=pr[:, 0:3], in_=th[:])
                nc.vector.tensor_scalar(out=pr[:, 3:4], in0=s[:], scalar1=127.5,
                                        scalar2=None, op0=ALU.add)
                nc.sync.dma_start(bass.AP(scr, b * 8, [[4, 2], [1, 4]]), pr[:])
                thb = wp.tile([P, 8], F32)
                nc.sync.dma_start(thb[:], bass.AP(scr, b * 8, [[0, P], [1, 8]]))

                it = wp.tile([P, 1536], F32)
                nc.sync.dma_start(it[:], bass.AP(inp, b * (6 + IMG_ELS) + 6,
                                                 [[1536, P], [1, 1536]]))
                hal = wp.tile([P, ROW_ELS], F32)
                nc.sync.dma_start(hal[0:127, :],
                                  bass.AP(inp, b * (6 + IMG_ELS) + 6 + 1536,
                                          [[1536, 127], [1, ROW_ELS]]))
                nc.sync.dma_start(hal[127:128, :],
                                  bass.AP(inp, b * (6 + IMG_ELS) + 6 + IMG_ELS - ROW_ELS,
                                          [[ROW_ELS, 1], [1, ROW_ELS]]))
                d2 = wp.tile([P, PW, 6], F32)
                it3 = it[:].rearrange("p (w c) -> p w c", c=3)
                nc.vector.tensor_copy(out=d2[:, :, 0:3], in_=it3)
                nc.vector.tensor_copy(out=d2[:, 0:256, 3:6],
                                      in_=it[:, ROW_ELS:1536].rearrange("p (w c) -> p w c", c=3))
                nc.vector.tensor_copy(out=d2[:, 256:512, 3:6],
                                      in_=hal[:].rearrange("p (w c) -> p w c", c=3))
                nc.sync.dma_start(idups[b][:, :], d2[:])

                X = wp.tile([P, PW], F32)
                nc.vector.tensor_scalar(out=X[:], in0=xg[:], scalar1=thb[:, 0:1],
                                        scalar2=None, op0=ALU.mult)
                X2 = wp.tile([P, PW], F32)
                nc.vector.scalar_tensor_tensor(out=X2[:], in0=yg[:], scalar=thb[:, 1:2],
                                               in1=X[:], op0=ALU.mult, op1=ALU.add)
                nc.vector.tensor_scalar(out=X[:], in0=X2[:], scalar1=thb[:, 3:4],
                                        scalar2=None, op0=ALU.add)
                Y = wp.tile([P, PW], F32)
                nc.vector.tensor_scalar(out=Y[:], in0=xg[:], scalar1=thb[:, 4:5],
                                        scalar2=None, op0=ALU.mult)
                Y2 = wp.tile([P, PW], F32)
                nc.vector.scalar_tensor_tensor(out=Y2[:], in0=yg[:], scalar=thb[:, 5:6],
                                               in1=Y[:], op0=ALU.mult, op1=ALU.add)
                nc.vector.tensor_scalar(out=Y[:], in0=Y2[:], scalar1=thb[:, 7:8],
                                        scalar2=None, op0=ALU.add)

                def floor_of(src, nm):
                    ti = wp.tile([P, PW], I32, tag=f"fl_i{nm}")
                    nc.vector.tensor_copy(out=ti[:], in_=src[:])
                    tf = wp.tile([P, PW], F32, tag=f"fl_f{nm}")
                    nc.vector.tensor_copy(out=tf[:], in_=ti[:])
                    gt = wp.tile([P, PW], F32, tag=f"fl_g{nm}")
                    nc.vector.tensor_tensor(out=gt[:], in0=tf[:], in1=src[:], op=ALU.is_gt)
                    fl = wp.tile([P, PW], F32, tag=f"fl_o{nm}")
                    nc.vector.tensor_tensor(out=fl[:], in0=tf[:], in1=gt[:], op=ALU.subtract)
                    return fl

                xf = floor_of(X, "x")
                yf = floor_of(Y, "y")

                fx = wp.tile([P, PW], F32)
                nc.vector.tensor_tensor(out=fx[:], in0=X[:], in1=xf[:], op=ALU.subtract)
                fy = wp.tile([P, PW], F32)
                nc.vector.tensor_tensor(out=fy[:], in0=Y[:], in1=yf[:], op=ALU.subtract)
                al = wp.tile([P, PW], F32)
                nc.vector.tensor_scalar(out=al[:], in0=fx[:], scalar1=-1.0, scalar2=1.0,
                                        op0=ALU.mult, op1=ALU.add)
                ga = wp.tile([P, PW], F32)
                nc.vector.tensor_scalar(out=ga[:], in0=fy[:], scalar1=-1.0, scalar2=1.0,
                                        op0=ALU.mult, op1=ALU.add)
                mgx = wp.tile([P, PW], F32)
                nc.vector.tensor_scalar(out=mgx[:], in0=xf[:], scalar1=0.0, scalar2=None,
                                        op0=ALU.is_ge)
                mx = wp.tile([P, PW], F32)
                nc.vector.scalar_tensor_tensor(out=mx[:], in0=xf[:], scalar=254.0,
                                               in1=mgx[:], op0=ALU.is_le, op1=ALU.mult)
                mgy = wp.tile([P, PW], F32)
                nc.vector.tensor_scalar(out=mgy[:], in0=yf[:], scalar1=0.0, scalar2=None,
                                        op0=ALU.is_ge)
                my = wp.tile([P, PW], F32)
                nc.vector.scalar_tensor_tensor(out=my[:], in0=yf[:], scalar=254.0,
                                               in1=mgy[:], op0=ALU.is_le, op1=ALU.mult)
                A = wp.tile([P, PW], F32)
                nc.vector.tensor_tensor(out=A[:], in0=al[:], in1=mx[:], op=ALU.mult)
                Bw = wp.tile([P, PW], F32)
                nc.vector.tensor_tensor(out=Bw[:], in0=fx[:], in1=mx[:], op=ALU.mult)
                Cw = wp.tile([P, PW], F32)
                nc.vector.tensor_tensor(out=Cw[:], in0=ga[:], in1=my[:], op=ALU.mult)
                Dw = wp.tile([P, PW], F32)
                nc.vector.tensor_tensor(out=Dw[:], in0=fy[:], in1=my[:], op=ALU.mult)
                w00 = wp.tile([P, PW], F32)
                nc.vector.tensor_tensor(out=w00[:], in0=Cw[:], in1=A[:], op=ALU.mult)
                w10 = wp.tile([P, PW], F32)
                nc.vector.tensor_tensor(out=w10[:], in0=Dw[:], in1=A[:], op=ALU.mult)
                w01 = wp.tile([P, PW], F32)
                nc.vector.tensor_tensor(out=w01[:], in0=Cw[:], in1=Bw[:], op=ALU.mult)
                w11 = wp.tile([P, PW], F32)
                nc.vector.tensor_tensor(out=w11[:], in0=Dw[:], in1=Bw[:], op=ALU.mult)

                xc = wp.tile([P, PW], F32)
                nc.vector.tensor_scalar(out=xc[:], in0=xf[:], scalar1=0.0, scalar2=254.0,
                                        op0=ALU.max, op1=ALU.min)
                yc = wp.tile([P, PW], F32)
                nc.vector.tensor_scalar(out=yc[:], in0=yf[:], scalar1=0.0, scalar2=254.0,
                                        op0=ALU.max, op1=ALU.min)
                lf = wp.tile([P, PW], F32)
                nc.vector.scalar_tensor_tensor(out=lf[:], in0=yc[:], scalar=256.0,
                                               in1=xc[:], op0=ALU.mult, op1=ALU.add)
                off = opool.tile([P, PW], I32)
                nc.vector.tensor_copy(out=off[:], in_=lf[:])

                g = gpool.tile([P, PW, 12], F32)
                for w in range(PW):
                    inst = nc.gpsimd.indirect_dma_start(
                        out=g[:, w, :], out_offset=None,
                        in_=idups[b][:, :],
                        in_offset=bass.IndirectOffsetOnAxis(ap=off[:, w:w + 1], axis=0))
                    if w % 2:
                        inst.ins.queue = "qPoolDynamic1"

                def bc3(t):
                    return bass.AP(t.tensor, t.offset, list(t.ap) + [[0, 3]])

                t0 = wp.tile([P, PW, 3], F32)
                nc.vector.tensor_tensor(out=t0[:], in0=g[:, :, 0:3], in1=bc3(w00[:]), op=ALU.mult)
                t1 = wp.tile([P, PW, 3], F32)
                nc.vector.tensor_tensor(out=t1[:], in0=g[:, :, 3:6], in1=bc3(w10[:]), op=ALU.mult)
                t2 = wp.tile([P, PW, 3], F32)
                nc.vector.tensor_tensor(out=t2[:], in0=g[:, :, 6:9], in1=bc3(w01[:]), op=ALU.mult)
                t3 = wp.tile([P, PW, 3], F32)
                nc.vector.tensor_tensor(out=t3[:], in0=g[:, :, 9:12], in1=bc3(w11[:]), op=ALU.mult)
                nc.vector.tensor_tensor(out=t0[:], in0=t0[:], in1=t1[:], op=ALU.add)
                nc.vector.tensor_tensor(out=t2[:], in0=t2[:], in1=t3[:], op=ALU.add)
                ob = wp.tile([P, PW, 3], F32)
                nc.vector.tensor_tensor(out=ob[:], in0=t0[:], in1=t2[:], op=ALU.add)
                nc.sync.dma_start(bass.AP(out_d, b * IMG_ELS, [[PW * 3, P], [1, PW * 3]]),
                                  ob[:])
    nc.compile()
    return nc


def _build(n_imgs):
    if GMODE == "dg":
        return _build_dg(n_imgs)
    return _build_i1(n_imgs)


def _consts():
    # pixel (p, w): global l = p*PW + w ; j = l % 256 ; i = l // 256
    l = np.arange(P * PW).reshape(P, PW)
    xg = (l % 256).astype(np.float32)
    yg = (l // 256).astype(np.float32)
    cst = np.tile(np.array([-127.5, -127.5, 127.5, 0.0], np.float32), (2, 1))
    i4 = np.tile(np.arange(4, dtype=np.float32), (P, 1))
    return xg, yg, cst, i4


IMGS_PER_LAUNCH = 16


def kernel(inputs: np.ndarray) -> np.ndarray:
    inputs = np.ascontiguousarray(inputs, dtype=np.float32)
    assert inputs.shape == (128, 6 + IMG_ELS)
    npl = IMGS_PER_LAUNCH
    key = (npl, GMODE)
    if key not in _cached:
        _cached[key] = _build(npl)
    nc = _cached[key]
    xg, yg, cst, i4 = _consts()
    trace = bool(os.environ.get("BILIN_TRACE"))
    if trace:
        try:  # NTFF trace hook is missing from this image's antenv; install shim
            import antenv.axon_hooks  # noqa: F401
        except ImportError:
            try:
                import types
                from trn_agent_boot.trn_boot import _ntff_profile_via_ctypes
                hook = _ntff_profile_via_ctypes("/opt/axon/libaxon_pjrt.so")
                mod = types.ModuleType("antenv.axon_hooks")
                mod.get_axon_ntff_profile_hook = lambda: hook
                sys.modules["antenv.axon_hooks"] = mod
            except Exception:
                trace = False
    out = np.empty((128, H, W, C), np.float32)
    total_ns = 0
    n_launches = IMGS // npl
    for k in range(n_launches):
        in_maps = []
        for c in range(N_CORES):
            lo = c * IMGS + k * npl
            in_maps.append(dict(inp=np.ascontiguousarray(inputs[lo:lo + npl]),
                                xg=xg, yg=yg, cst=cst, i4=i4))
        res = run_bass_kernel_spmd(nc, in_maps, core_ids=list(range(N_CORES)),
                                   trace=trace and k == 0)
        if trace and k == 0 and res.exec_time_ns is not None:
            total_ns = res.exec_time_ns * n_launches
        for c in range(N_CORES):
            lo = c * IMGS + k * npl
            out[lo:lo + npl] = res.results[c]["out"].reshape(npl, H, W, C)
    if trace:
        print(f"HW exec time: {total_ns} ns")
    return out
